# revision 1
# baseline (speedup 1.0000x reference)
"""Trainium2 Bass kernel for LGAttention (global MHA + windowed local MHA).

Sharding: one attention head per NeuronCore (8 heads, 8 cores), SPMD.
Each core computes, for its head h:
  - global branch: q/k/v projections, flash-style softmax(q k^T)·v in
    "S^T layout" (k on partitions, q on free). The PV matmul uses v augmented
    with a ones column at col 64 (cols 48-63 zero-padded so the softmax
    denominator lands on partition 64, a legal base partition), producing
    unnormalized out^T plus the denominator in one pass.
  - local branch: same for the 128 independent 49-token windows.
  - output projection with the head's 48-row slice of proj_w (unnormalized).
Host: divides by the denominators, un-permutes the local branch, sums the
8 per-head partials, adds biases.
"""

import sys

sys.path.insert(0, "/opt/trn_rl_repo")

import numpy as np
import ml_dtypes

import concourse.bass as bass
import concourse.mybir as mybir
import concourse.tile as tile
from concourse import bacc, bass_utils

BF16 = mybir.dt.bfloat16
F32 = mybir.dt.float32

B, N, C = 2, 3136, 384
H, HD, WS = 8, 48, 7
NT = B * N            # 6272 tokens total
WT = WS * WS          # 49 tokens per window
QB = 448              # q-tile (free dim) for global attention
VS = 65               # v_aug column stride: 48 v + 16 pad + 1 ones
SCALE = float(HD) ** -0.5


def build_program():
    nc = bacc.Bacc(
        "TRN2",
        target_bir_lowering=False,
        debug=False,
        enable_asserts=False,
        num_devices=8,
    )

    din = {}
    for name, shape in [
        ("xT", (C, NT)), ("winT", (C, NT)),
        ("gwqk", (C, 112)), ("gwv", (C, HD)), ("gwp", (HD, C)),
        ("lwqk", (C, 112)), ("lwv", (C, HD)), ("lwp", (HD, C)),
    ]:
        din[name] = nc.dram_tensor(name, list(shape), BF16, kind="ExternalInput").ap()

    dout = {}
    for name, shape in [
        ("g_out", (NT, C)), ("l_out", (NT, C)),
        ("g_den", (1, NT)), ("l_den", (1, NT)),
    ]:
        dout[name] = nc.dram_tensor(name, list(shape), F32, kind="ExternalOutput").ap()

    with tile.TileContext(nc) as tc:
        _emit(tc, nc, din, dout)

    nc.compile()
    return nc


def _emit(tc, nc, din, dout):
    from contextlib import ExitStack

    ctx = ExitStack()
    with ctx:
        persist = ctx.enter_context(tc.tile_pool(name="persist", bufs=1))
        psum = ctx.enter_context(tc.tile_pool(name="psum", bufs=2, space="PSUM"))
        work = ctx.enter_context(tc.tile_pool(name="work", bufs=3))

        # ---- load inputs to SBUF ----
        xt = [persist.tile([128, NT], BF16, name=f"xt{c}") for c in range(3)]
        wt = [persist.tile([128, NT], BF16, name=f"wt{c}") for c in range(3)]
        for c in range(3):
            nc.sync.dma_start(xt[c][:, :], din["xT"][c * 128:(c + 1) * 128, :])
            nc.sync.dma_start(wt[c][:, :], din["winT"][c * 128:(c + 1) * 128, :])
        gwqk = persist.tile([128, 3 * 112], BF16, name="gwqk")
        lwqk = persist.tile([128, 3 * 112], BF16, name="lwqk")
        gwv = persist.tile([128, 3 * 48], BF16, name="gwv")
        lwv = persist.tile([128, 3 * 48], BF16, name="lwv")
        for c in range(3):
            nc.sync.dma_start(gwqk[:, c * 112:(c + 1) * 112], din["gwqk"][c * 128:(c + 1) * 128, :])
            nc.sync.dma_start(lwqk[:, c * 112:(c + 1) * 112], din["lwqk"][c * 128:(c + 1) * 128, :])
            nc.sync.dma_start(gwv[:, c * 48:(c + 1) * 48], din["gwv"][c * 128:(c + 1) * 128, :])
            nc.sync.dma_start(lwv[:, c * 48:(c + 1) * 48], din["lwv"][c * 128:(c + 1) * 128, :])
        gwp = persist.tile([HD, C], BF16, name="gwp")
        lwp = persist.tile([HD, C], BF16, name="lwp")
        nc.sync.dma_start(gwp[:, :], din["gwp"][:, :])
        nc.sync.dma_start(lwp[:, :], din["lwp"][:, :])

        # ---- persistent intermediates ----
        g_qT = persist.tile([HD, NT], BF16, name="g_qT")
        g_kT = persist.tile([HD, NT], BF16, name="g_kT")
        l_qT = persist.tile([HD, NT], BF16, name="l_qT")
        l_kT = persist.tile([HD, NT], BF16, name="l_kT")
        g_vaug = persist.tile([128, 50 * VS], BF16, name="g_vaug")  # 25 kb-blocks/batch
        l_vaug = persist.tile([49, 128 * VS], BF16, name="l_vaug")  # one block per window
        g_outT = persist.tile([HD, NT], BF16, name="g_outT")
        l_outT = persist.tile([HD, NT], BF16, name="l_outT")

        # v_aug pad/ones columns (softmax denominator comes out of the PV matmul)
        nc.vector.memset(g_vaug[:, :].rearrange("p (b k) -> p b k", k=VS)[:, :, 48:VS], 0.0)
        nc.vector.memset(l_vaug[:, :].rearrange("p (b k) -> p b k", k=VS)[:, :, 48:VS], 0.0)
        nc.vector.memset(g_vaug[:, :].rearrange("p (b k) -> p b k", k=VS)[:, :, 64:VS], 1.0)
        nc.vector.memset(l_vaug[:, :].rearrange("p (b k) -> p b k", k=VS)[:, :, 64:VS], 1.0)

        # ---- q/k projections: psum rows 0-47 = q, 64-111 = k (zero gap in W) ----
        for src, qT, kT, wqk in ((xt, g_qT, g_kT, gwqk), (wt, l_qT, l_kT, lwqk)):
            for qb in range(14):
                t0 = qb * QB
                ps = psum.tile([112, QB], F32, name="pqk", tag="pmix", bufs=4)
                for c in range(3):
                    nc.tensor.matmul(ps[:, :], wqk[:, c * 112:(c + 1) * 112],
                                     src[c][:, t0:t0 + QB], start=(c == 0), stop=(c == 2))
                nc.vector.tensor_copy(qT[:, t0:t0 + QB], ps[0:48, :])
                nc.vector.tensor_copy(kT[:, t0:t0 + QB], ps[64:112, :])

        # ---- v projections (token-major) ----
        for b in range(2):
            for j in range(25):
                sz = 128 if j < 24 else 64
                t0 = b * N + j * 128
                bl = b * 25 + j
                ps = psum.tile([128, HD], F32, name="pv", tag="pmix", bufs=4)
                for c in range(3):
                    nc.tensor.matmul(ps[0:sz, :], xt[c][:, t0:t0 + sz],
                                     gwv[:, c * 48:(c + 1) * 48], start=(c == 0), stop=(c == 2))
                nc.vector.tensor_copy(g_vaug[0:sz, bl * VS:bl * VS + 48], ps[0:sz, :])
        for w in range(64):
            t0 = w * 2 * WT
            ps = psum.tile([128, 2 * HD], F32, name="pvl", tag="pmix", bufs=4)
            for c in range(3):
                nc.tensor.matmul(ps[0:WT, 0:HD], wt[c][:, t0:t0 + WT],
                                 lwv[:, c * 48:(c + 1) * 48], start=(c == 0), stop=(c == 2))
            for c in range(3):
                nc.tensor.matmul(ps[0:WT, HD:2 * HD], wt[c][:, t0 + WT:t0 + 2 * WT],
                                 lwv[:, c * 48:(c + 1) * 48], start=(c == 0), stop=(c == 2))
            nc.vector.tensor_copy(l_vaug[0:WT, (2 * w) * VS:(2 * w) * VS + 48], ps[0:WT, 0:HD])
            nc.vector.tensor_copy(l_vaug[0:WT, (2 * w + 1) * VS:(2 * w + 1) * VS + 48], ps[0:WT, HD:2 * HD])

        # ---- local attention first: 16 groups of 8 windows ----
        for grp in range(16):
            psl = psum.tile([49, 392], F32, name="pSl", tag="pmix", bufs=4)
            for w8 in range(8):
                w = grp * 8 + w8
                t0 = w * WT
                nc.tensor.matmul(psl[:, w8 * WT:(w8 + 1) * WT],
                                 l_kT[:, t0:t0 + WT], l_qT[:, t0:t0 + WT],
                                 start=True, stop=True)
            exl = work.tile([49, 392], BF16, name="expSl")
            nc.scalar.activation(exl[:, :], psl[:, :],
                                 mybir.ActivationFunctionType.Exp, scale=SCALE)
            pol = psum.tile([VS, 392], F32, name="poutl", tag="pmix", bufs=4)
            for w8 in range(8):
                w = grp * 8 + w8
                nc.tensor.matmul(pol[:, w8 * WT:(w8 + 1) * WT],
                                 l_vaug[0:WT, w * VS:w * VS + VS],
                                 exl[:, w8 * WT:(w8 + 1) * WT], start=True, stop=True)
            nc.vector.tensor_copy(l_outT[:, grp * 392:(grp + 1) * 392], pol[0:48, :])
            dnl = work.tile([1, 392], F32, name="dnl", tag="dn", bufs=3)
            nc.vector.tensor_copy(dnl[:, :], pol[64:VS, :])
            nc.sync.dma_start(dout["l_den"][0:1, grp * 392:(grp + 1) * 392], dnl[:, :])

        # ---- global attention: qb pairs, one 896-wide exp per two S matmuls,
        # PV software-pipelined one kb iteration behind S so PE never stalls ----
        for b in range(2):
            for qp in range(4):
                qw = 448 if qp == 3 else 896
                nsub = qw // QB
                q0 = b * N + qp * 896
                po = [psum.tile([VS, QB], F32, name=f"po{s}", tag="pmix", bufs=4)
                      for s in range(nsub)]
                exs = [None] * 25
                for j in range(26):
                    if j < 25:
                        sz = 128 if j < 24 else 64
                        k0 = b * N + j * 128
                        ps = psum.tile([128, 1024], F32, name="pS", tag="pS", bufs=2)
                        for s in range(nsub):
                            nc.tensor.matmul(ps[0:sz, s * 512:s * 512 + QB],
                                             g_kT[:, k0:k0 + sz],
                                             g_qT[:, q0 + s * QB:q0 + (s + 1) * QB],
                                             start=True, stop=True)
                        ex = work.tile([128, 896], BF16, name="expS")
                        ps_v = ps[0:sz, :].rearrange("p (u k) -> p u k", k=512)[:, 0:nsub, 0:QB]
                        ex_v = ex[0:sz, 0:qw].rearrange("p (u k) -> p u k", k=QB)
                        nc.scalar.activation(ex_v, ps_v,
                                             mybir.ActivationFunctionType.Exp, scale=SCALE)
                        exs[j] = (ex, sz)
                    if j >= 1:
                        jj = j - 1
                        ex, sz = exs[jj]
                        bl = b * 25 + jj
                        for s in range(nsub):
                            nc.tensor.matmul(po[s][:, :],
                                             g_vaug[0:sz, bl * VS:bl * VS + VS],
                                             ex[0:sz, s * QB:(s + 1) * QB],
                                             start=(jj == 0), stop=(jj == 24))
                for s in range(nsub):
                    q0s = q0 + s * QB
                    nc.vector.tensor_copy(g_outT[:, q0s:q0s + QB], po[s][0:48, :])
                    dn = work.tile([1, QB], F32, name="dn", tag="dn", bufs=3)
                    nc.vector.tensor_copy(dn[:, :], po[s][64:VS, :])
                    nc.sync.dma_start(dout["g_den"][0:1, q0s:q0s + QB], dn[:, :])
                # interleave output projection for this region (l_outT is complete)
                for blk in range(qw // 112):
                    t0 = q0 + blk * 112
                    for outT, wp, dst in ((g_outT, gwp, dout["g_out"]),
                                          (l_outT, lwp, dout["l_out"])):
                        pp = psum.tile([112, C], F32, name="pp", tag="pmix", bufs=4)
                        nc.tensor.matmul(pp[:, :], outT[:, t0:t0 + 112], wp[:, :],
                                         start=True, stop=True)
                        sp = work.tile([112, C], F32, name="sproj", tag="sproj", bufs=4)
                        nc.vector.tensor_copy(sp[:, :], pp[:, :])
                        nc.sync.dma_start(dst[t0:t0 + 112, :], sp[:, :])


def _host_prep(x, g_qkv_w, g_proj_w, l_qkv_w, l_proj_w):
    bf = ml_dtypes.bfloat16
    xf = np.asarray(x, np.float32).reshape(NT, C)
    xT = np.ascontiguousarray(xf.T).astype(bf)
    x4 = np.asarray(x, np.float32).reshape(B, 56, 56, C)
    win = x4.reshape(B, 8, WS, 8, WS, C).transpose(0, 1, 3, 5, 2, 4)
    win = win.reshape(B, 8, 8, WS, WS, C).transpose(0, 1, 2, 4, 3, 5).reshape(NT, C)
    winT = np.ascontiguousarray(win.T).astype(bf)

    in_maps = []
    for h in range(8):
        m = {"xT": xT, "winT": winT}
        for pre, qkv_w, proj_w in (("g", g_qkv_w, g_proj_w), ("l", l_qkv_w, l_proj_w)):
            qw = np.asarray(qkv_w[:, h * HD:(h + 1) * HD], np.float32)
            kw = np.asarray(qkv_w[:, C + h * HD:C + (h + 1) * HD], np.float32)
            vw = np.asarray(qkv_w[:, 2 * C + h * HD:2 * C + (h + 1) * HD], np.float32)
            wqk = np.zeros((C, 112), np.float32)
            wqk[:, 0:48] = qw
            wqk[:, 64:112] = kw
            m[pre + "wqk"] = wqk.astype(bf)
            m[pre + "wv"] = np.ascontiguousarray(vw).astype(bf)
            m[pre + "wp"] = np.ascontiguousarray(
                np.asarray(proj_w, np.float32)[h * HD:(h + 1) * HD, :]).astype(bf)
        in_maps.append(m)
    return in_maps


_NC_CACHE = None


def kernel(x, g_qkv_w, g_proj_w, g_proj_b, l_qkv_w, l_proj_w, l_proj_b):
    global _NC_CACHE
    if _NC_CACHE is None:
        _NC_CACHE = build_program()
    nc = _NC_CACHE

    in_maps = _host_prep(x, g_qkv_w, g_proj_w, l_qkv_w, l_proj_w)
    res = bass_utils.run_bass_kernel_spmd(nc, in_maps, core_ids=list(range(8)))

    acc = np.zeros((NT, C), np.float32)
    l_acc = np.zeros((NT, C), np.float32)
    for h in range(8):
        r = res.results[h]
        acc += np.asarray(r["g_out"], np.float32) / np.asarray(r["g_den"], np.float32).reshape(NT, 1)
        l_acc += np.asarray(r["l_out"], np.float32) / np.asarray(r["l_den"], np.float32).reshape(NT, 1)
    l_tok = l_acc.reshape(B, 8, 8, WS, WS, C).transpose(0, 1, 3, 2, 4, 5).reshape(NT, C)
    out = acc + l_tok + np.asarray(g_proj_b, np.float32) + np.asarray(l_proj_b, np.float32)
    return out.reshape(B, N, C).astype(np.float32)

